# revision 25
# baseline (speedup 1.0000x reference)
"""Fused 7-gate continuous-time LSTM cell on 8 Trainium2 NeuronCores.

Data-parallel over the batch dim: each core gets B/8 = 1024 rows, the fused
gate weight W [2048, 7*2048] is replicated. g = hx @ W + b runs in mixed
precision, chosen so the worst-case output error stays ~27% under the 2e-2
gate (validated against the fp32 reference in numpy):
  - sigmoid gates (i1,i2,f1,f2,o): K rows 0..1535 in bf16, rows 1536..2047
    in fp8e4m3 with DoubleRow (2 MACs/cell/cycle)
  - z (tanh) gate: all bf16 (most error-sensitive)
  - d (decay) gate: all fp8 DoubleRow (its quantization error is invisible
    in the output: softplus->exp(-u*decay) damps it)
All weights are host-scaled x64 so fp8 W values sit in e4m3's normal range;
the /64 is folded into the ACT `scale` operand of the epilogue (free).
The z gate is additionally pre-scaled x2 so tanh(x) = 2*sigmoid(2x)-1 rides
the batched sigmoid chain.

Epilogue uses only Exp/Ln (single ACT table set, no ~2.7us table thrash):
  sigmoid(x) = exp(-softplus(-x)) = Exp(-Ln(1+Exp(-x)))
  E          = exp(-u*softplus(d)) via Exp(scale=-u[partition]) on Ln(1+Exp(d))

Per-block work is software-pipelined in two phases so the in-order DVE queue
never holds block i's gate arithmetic (which waits on ACT) ahead of block
i+1's PSUM-draining bias-add (which ACT i+1 needs):
  phase1(i) = matmuls, bias-add, batched sigmoid chain, E chain
  phase2(i) = gate arithmetic, tanh(c_t) chain, stores  (after phase1(i+1))
"""

import sys

sys.path.insert(0, "/opt/trn_rl_repo")

import numpy as np
import ml_dtypes

import concourse.bass as bass
import concourse.mybir as mybir
import concourse.tile as tile
from concourse import bacc, bass_utils

B, D, H, NG = 8192, 2048, 2048, 7
N_CORES = 8
BL = B // N_CORES  # 1024 rows per core
P = 128
HB = 256  # h-block per epilogue step
N_HB = H // HB  # 8
GW = NG * HB  # 1792 gate-block width per hb
KT = D // P  # 16 contraction subtiles
MT = BL // P  # 8 m-chunks per core
KBF = 12  # bf16 k-subtiles for the sigmoid gates (k < 1536)
SW = 6 * HB  # sigmoid+z block width in gsb (1536)

F32 = mybir.dt.float32
BF16 = mybir.dt.bfloat16
FP8 = mybir.dt.float8e4
AF = mybir.ActivationFunctionType
DR = mybir.MatmulPerfMode.DoubleRow
BF16NP = ml_dtypes.bfloat16
FP8NP = ml_dtypes.float8_e4m3

_cached_nc = None

# Make Exp/Ln resolvable only from the one ACT table set that holds both, so
# the table-load inserter hoists a single ACT_TABLE_LOAD instead of thrashing
# ~3 loads (~1.3us each) per block between an exp-set and an ln-set. Set order
# and count are preserved so act_func_set_id indices stay valid.
_ACT_SET = "natural_log_exp_and_others"
_tables_patched = False


def _patch_act_tables():
    global _tables_patched
    if _tables_patched:
        return
    orig = bacc.get_activation_tables

    def patched(arch):
        tabs = {k: set(v) for k, v in orig(arch).items()}
        assert _ACT_SET in tabs and {AF.Exp, AF.Ln} <= tabs[_ACT_SET], tabs.keys()
        for k in tabs:
            if k != _ACT_SET:
                tabs[k] -= {AF.Exp, AF.Ln}
        return tabs

    bacc.get_activation_tables = patched
    _tables_patched = True


def _build():
    _patch_act_tables()
    nc = bacc.Bacc("TRN2", target_bir_lowering=False, debug=False,
                   num_devices=N_CORES)
    hxT = nc.dram_tensor("hxT", [D, BL], BF16, kind="ExternalInput").ap()
    hx8T = nc.dram_tensor("hx8T", [D, BL], FP8, kind="ExternalInput").ap()
    cx1 = nc.dram_tensor("cx1", [BL, H], F32, kind="ExternalInput").ap()
    cx2 = nc.dram_tensor("cx2", [BL, H], F32, kind="ExternalInput").ap()
    dt_in = nc.dram_tensor("dt", [BL, 1], F32, kind="ExternalInput").ap()
    # bf16 weights: k<1536 x [hb, 6 gates(i1,i2,f1,f2,o,z), 256]
    Wa = nc.dram_tensor("Wa", [KBF * P, N_HB * SW], BF16,
                        kind="ExternalInput").ap()
    # bf16 o+z gate tail: k>=1536 x [hb, 512]
    Wb = nc.dram_tensor("Wb", [(KT - KBF) * P, N_HB * 512], BF16,
                        kind="ExternalInput").ap()
    # fp8 sigmoid-gate tail, one contiguous block per DR matmul slice:
    # [t'(2), hb, p, j, w] with k = 1536 + t'*256 + j*128 + p
    W8a = nc.dram_tensor("W8a", [2, N_HB, P, 2, 512], FP8,
                         kind="ExternalInput").ap()
    W8b = nc.dram_tensor("W8b", [2, N_HB, P, 2, 512], FP8,
                         kind="ExternalInput").ap()
    # fp8 d gate, all k: [t(8), hb, p, j, 256]
    W8d = nc.dram_tensor("W8d", [8, N_HB, P, 2, HB], FP8,
                         kind="ExternalInput").ap()
    bp = nc.dram_tensor("bp", [N_HB, GW], BF16, kind="ExternalInput").ap()
    out = nc.dram_tensor("out", [3, BL, H], F32, kind="ExternalOutput").ap()

    from contextlib import ExitStack

    with tile.TileContext(nc) as tc, ExitStack() as ctx:
        const_pool = ctx.enter_context(tc.tile_pool(name="const", bufs=1))
        hx_pool = ctx.enter_context(tc.tile_pool(name="hx", bufs=1))
        small_pool = ctx.enter_context(tc.tile_pool(name="small", bufs=4))
        wa_pool = ctx.enter_context(tc.tile_pool(name="wa", bufs=17))
        wb_pool = ctx.enter_context(tc.tile_pool(name="wb", bufs=6))
        w8s_pool = ctx.enter_context(tc.tile_pool(name="w8s", bufs=3))
        w8d_pool = ctx.enter_context(tc.tile_pool(name="w8d", bufs=10))
        bias_pool = ctx.enter_context(tc.tile_pool(name="bias", bufs=2))
        psum_pool = ctx.enter_context(tc.tile_pool(name="ps", bufs=4, space="PSUM"))
        gsb_pool = ctx.enter_context(tc.tile_pool(name="gsb", bufs=2))
        sgt_pool = ctx.enter_context(tc.tile_pool(name="sgt", bufs=2))
        sig_pool = ctx.enter_context(tc.tile_pool(name="sig", bufs=2))
        epi_pool = ctx.enter_context(tc.tile_pool(name="epi", bufs=2))
        cx_pool = ctx.enter_context(tc.tile_pool(name="cx", bufs=4))
        out_pool = ctx.enter_context(tc.tile_pool(name="outp", bufs=2))

        # -u per batch row (u = dt), laid out [128, m-chunk]
        negu = const_pool.tile([P, MT], F32)
        for m in range(MT):
            dtt = small_pool.tile([P, 1], F32, tag="dt")
            nc.scalar.dma_start(dtt, dt_in[m * P : (m + 1) * P, :])
            nc.vector.tensor_scalar_mul(negu[:, m : m + 1], dtt, -1.0)

        # resident hx^T in k-chunked tiles, issued in first-use order so the
        # first matmuls wait on ~1MB, not the whole 6MB
        def _hx_load(dram, dtype, base, n, tag):
            t = hx_pool.tile([P, n, BL], dtype, tag=tag, name=f"hx_{tag}")
            nc.gpsimd.dma_start(
                t,
                bass.AP(tensor=dram.tensor, offset=dram.offset + base * P * BL,
                        ap=[[BL, P], [P * BL, n], [1, BL]]),
            )
            return (base, n, t)
        hxT_tiles = [_hx_load(hxT, BF16, 0, 4, "t0")]
        hxT_tiles.append(_hx_load(hxT, BF16, 4, 8, "t4"))
        hx8_tiles = [_hx_load(hx8T, FP8, 0, 12, "e0")]
        hxT_tiles.append(_hx_load(hxT, BF16, 12, 4, "t12"))
        hx8_tiles.append(_hx_load(hx8T, FP8, 12, 4, "e12"))

        def hxT_lhs(j, ms):
            for base, n, t in hxT_tiles:
                if base <= j < base + n:
                    return t[:, j - base, ms]

        def hx8_lhs(ksub, ms):
            for base, n, t in hx8_tiles:
                if base <= ksub < base + n - 1:
                    return t[:, ksub - base : ksub - base + 2, ms]

        def phase1(hb, m, bt, wa, wb, w8s, w8d):
            ms = slice(m * P, (m + 1) * P)
            cs = slice(hb * HB, (hb + 1) * HB)
            cx1t = cx_pool.tile([P, HB], F32, tag="cx1")
            nc.gpsimd.dma_start(cx1t, cx1[ms, cs])
            cx2t = cx_pool.tile([P, HB], F32, tag="cx2")
            nc.gpsimd.dma_start(cx2t, cx2[ms, cs])

            ps0 = psum_pool.tile([P, 1024], F32, tag="ps", name=f"ps0_{hb}_{m}")
            ps1 = psum_pool.tile([P, 1024], F32, tag="ps", name=f"ps1_{hb}_{m}")
            # gsb col ranges: ps0 -> [i1,i2,f1,f2], ps1[0:256] -> o,
            # ps1[256:512] -> z, ps1[512:768] -> d
            for j in range(KBF):
                lhs = hxT_lhs(j, ms)
                st = j == 0
                nc.tensor.matmul(ps0[:, 0:512], lhs, wa[j][:, 0:512],
                                 start=st, stop=False)
                nc.tensor.matmul(ps0[:, 512:1024], lhs, wa[j][:, 512:1024],
                                 start=st, stop=False)
                nc.tensor.matmul(ps1[:, 0:512], lhs, wa[j][:, 1024:1536],
                                 start=st, stop=False)
                if j < 6:  # d-gate fp8 DoubleRow interleaved: LDW hides
                    nc.tensor.matmul(ps1[:, 512:768], hx8_lhs(2 * j, ms),
                                     w8d[j][:], start=(j == 0), stop=False,
                                     perf_mode=DR)
            for j in range(KBF, KT):  # o+z gate bf16 tail
                nc.tensor.matmul(ps1[:, 0:512], hxT_lhs(j, ms),
                                 wb[j - KBF][:], start=False, stop=(j == KT - 1))
            for t in (6, 7):  # i1,i2,f1,f2 + d-gate fp8 DoubleRow tail
                l8 = hx8_lhs(2 * t, ms)
                last = t == 7
                w8at, w8bt = w8s[t - 6]
                nc.tensor.matmul(ps0[:, 0:512], l8, w8at[:],
                                 start=False, stop=last, perf_mode=DR)
                nc.tensor.matmul(ps0[:, 512:1024], l8, w8bt[:],
                                 start=False, stop=last, perf_mode=DR)
                nc.tensor.matmul(ps1[:, 512:768], l8, w8d[t][:],
                                 start=False, stop=last, perf_mode=DR)

            # bias add drains PSUM; gsb = 64*(g+b) (weights/bias host-scaled)
            gsb = gsb_pool.tile([P, GW], F32, tag="gsb")
            nc.vector.tensor_add(gsb[:, 0:1024], ps0[:], bt[:, 0:1024])
            nc.vector.tensor_add(gsb[:, 1024:GW], ps1[:, 0:768], bt[:, 1024:GW])

            # 5 sigmoid gates + pre-scaled z gate batched 1536 wide;
            # the 1/64 unscale rides the first Exp: sig = Exp(-Ln(1+Exp(-x/64)))
            e1 = sgt_pool.tile([P, SW], F32, tag="sgt")
            nc.scalar.activation(e1, gsb[:, 0:SW], AF.Exp, scale=-1.0 / 64)
            sp = sgt_pool.tile([P, SW], F32, tag="sgt")
            nc.scalar.activation(sp, e1, AF.Ln, bias=1.0)
            sig = sig_pool.tile([P, SW], F32, tag="sig")
            nc.scalar.activation(sig, sp, AF.Exp, scale=-1.0)

            # E = exp(-u * softplus(d))
            ed = epi_pool.tile([P, HB], F32, tag="ed")
            nc.scalar.activation(ed, gsb[:, 6 * HB : 7 * HB], AF.Exp,
                                 scale=1.0 / 64)
            spd = epi_pool.tile([P, HB], F32, tag="spd")
            nc.scalar.activation(spd, ed, AF.Ln, bias=1.0)
            E = epi_pool.tile([P, HB], F32, tag="E")
            nc.scalar.activation(E, spd, AF.Exp, scale=negu[:, m : m + 1])

            return dict(ms=ms, cs=cs, cx1t=cx1t, cx2t=cx2t, sig=sig, E=E)

        def phase2(st):
            sig, E = st["sig"], st["E"]
            i1 = sig[:, 0:HB]
            i2 = sig[:, HB : 2 * HB]
            f1 = sig[:, 2 * HB : 3 * HB]
            f2 = sig[:, 3 * HB : 4 * HB]
            o = sig[:, 4 * HB : 5 * HB]
            # z = tanh = 2*sigmoid(2x)-1 (the x2 was folded into W/b on host)
            z = epi_pool.tile([P, HB], F32, tag="z")
            nc.vector.tensor_scalar(
                out=z, in0=sig[:, 5 * HB : 6 * HB], scalar1=2.0, scalar2=-1.0,
                op0=mybir.AluOpType.mult, op1=mybir.AluOpType.add,
            )

            t1 = epi_pool.tile([P, HB], F32, tag="t1")
            nc.vector.tensor_mul(t1, f1, st["cx1t"])
            t2 = epi_pool.tile([P, HB], F32, tag="t2")
            nc.vector.tensor_mul(t2, i1, z)
            cy1 = out_pool.tile([P, HB], F32, tag="cy1")
            nc.vector.tensor_add(cy1, t1, t2)

            t3 = epi_pool.tile([P, HB], F32, tag="t3")
            nc.vector.tensor_mul(t3, f2, st["cx2t"])
            t4 = epi_pool.tile([P, HB], F32, tag="t4")
            nc.vector.tensor_mul(t4, i2, z)
            cy2 = out_pool.tile([P, HB], F32, tag="cy2")
            nc.vector.tensor_add(cy2, t3, t4)

            dif = epi_pool.tile([P, HB], F32, tag="dif")
            nc.vector.tensor_sub(dif, cy1, cy2)
            t5 = epi_pool.tile([P, HB], F32, tag="t5")
            nc.vector.tensor_mul(t5, dif, E)
            ct = epi_pool.tile([P, HB], F32, tag="ct")
            nc.vector.tensor_add(ct, cy2, t5)

            # tanh(ct) = 2*sigmoid(2*ct) - 1
            ec = epi_pool.tile([P, HB], F32, tag="ec")
            nc.scalar.activation(ec, ct, AF.Exp, scale=-2.0)
            spc = epi_pool.tile([P, HB], F32, tag="spc")
            nc.scalar.activation(spc, ec, AF.Ln, bias=1.0)
            s2c = epi_pool.tile([P, HB], F32, tag="s2c")
            nc.scalar.activation(s2c, spc, AF.Exp, scale=-1.0)
            tct = epi_pool.tile([P, HB], F32, tag="tct")
            nc.vector.tensor_scalar(
                out=tct, in0=s2c, scalar1=2.0, scalar2=-1.0,
                op0=mybir.AluOpType.mult, op1=mybir.AluOpType.add,
            )
            ht = out_pool.tile([P, HB], F32, tag="ht")
            nc.vector.tensor_mul(ht, o, tct)

            nc.gpsimd.dma_start(out[0, st["ms"], st["cs"]], cy1)
            nc.gpsimd.dma_start(out[1, st["ms"], st["cs"]], cy2)
            nc.gpsimd.dma_start(out[2, st["ms"], st["cs"]], ht)

        def issue_hb_loads(hb):
            bsl = bp[hb, :]
            b_bcast = bass.AP(
                tensor=bsl.tensor, offset=bsl.offset, ap=[[0, P], *bsl.ap]
            )
            bt = bias_pool.tile([P, GW], BF16, tag="bt", name=f"bt_{hb}")
            nc.gpsimd.dma_start(bt, b_bcast)
            wa, wb, w8s, w8d = [], [], [], []
            for j in range(KT - KBF):
                wt = wb_pool.tile([P, 512], BF16, tag="wb", name=f"wb_{hb}_{j}")
                nc.sync.dma_start(
                    wt, Wb[j * P : (j + 1) * P, hb * 512 : (hb + 1) * 512]
                )
                wb.append(wt)
            for t in range(2):
                w8at = w8s_pool.tile([P, 2, 512], FP8, tag="w8a",
                                     name=f"w8a_{hb}_{t}")
                nc.sync.dma_start(w8at, W8a[t, hb])
                w8bt = w8s_pool.tile([P, 2, 512], FP8, tag="w8b",
                                     name=f"w8b_{hb}_{t}")
                nc.sync.dma_start(w8bt, W8b[t, hb])
                w8s.append((w8at, w8bt))
            for t in range(8):
                w8dt = w8d_pool.tile([P, 2, HB], FP8, tag="w8d",
                                     name=f"w8d_{hb}_{t}")
                nc.sync.dma_start(w8dt, W8d[t, hb])
                w8d.append(w8dt)
            return bt, wb, w8s, w8d

        def issue_wa_tiles(hb, js):
            res = []
            for j in js:
                wt = wa_pool.tile([P, SW], BF16, tag="wa", name=f"wa_{hb}_{j}")
                nc.sync.dma_start(
                    wt, Wa[j * P : (j + 1) * P, hb * SW : (hb + 1) * SW]
                )
                res.append(wt)
            return res

        pending = None
        wa_head = issue_wa_tiles(0, range(4))
        for hb in range(N_HB):
            wa = wa_head + issue_wa_tiles(hb, range(4, KBF))
            bt, wb, w8s, w8d = issue_hb_loads(hb)
            for m in range(MT):
                st = phase1(hb, m, bt, wa, wb, w8s, w8d)
                if m == 6 and hb + 1 < N_HB:
                    wa_head = issue_wa_tiles(hb + 1, range(4))
                if pending is not None:
                    phase2(pending)
                pending = st
        phase2(pending)

    nc.compile()
    return nc


def _get_nc():
    global _cached_nc
    if _cached_nc is None:
        _cached_nc = _build()
    return _cached_nc


def kernel(hx, cx1, cx2, tj, dt, W, b, trace=False):
    nc = _get_nc()
    Wm = np.asarray(W, dtype=np.float32).copy()
    bm = np.asarray(b, dtype=np.float32).reshape(NG * H).copy()
    # fold tanh(x) = 2*sigmoid(2x)-1: pre-scale z-gate columns by 2
    Wm[:, 5 * H : 6 * H] *= 2.0
    bm[5 * H : 6 * H] *= 2.0
    # global x64 so fp8 weights sit in e4m3's normal range; /64 folded into
    # the epilogue ACT scale
    Wm *= 64.0
    bm *= 64.0
    W4 = Wm.reshape(D, NG, N_HB, HB)  # [k, gate, hb, col]
    Wa_np = np.ascontiguousarray(
        W4[: KBF * P, :6].transpose(0, 2, 1, 3).reshape(KBF * P, N_HB * SW)
        .astype(BF16NP)
    )
    Wb_np = np.ascontiguousarray(
        W4[KBF * P :, 4:6].transpose(0, 2, 1, 3)
        .reshape((KT - KBF) * P, N_HB * 512).astype(BF16NP)
    )
    q8 = lambda a: np.clip(a, -240, 240).astype(FP8NP)
    W8s_full = q8(W4[KBF * P :, :4]
                  .transpose(0, 2, 1, 3)           # [k', hb, gate, col]
                  .reshape(2, 2, P, N_HB, 4 * HB)  # [t', j, p, hb, c]
                  .transpose(0, 3, 2, 1, 4))       # [t', hb, p, j, c]
    W8a_np = np.ascontiguousarray(W8s_full[..., 0:512])
    W8b_np = np.ascontiguousarray(W8s_full[..., 512:1024])
    W8d_np = np.ascontiguousarray(
        q8(W4[:, 6]
           .reshape(8, 2, P, N_HB, HB)        # [t, j, p, hb, c]
           .transpose(0, 3, 2, 1, 4))         # [t, hb, p, j, c]
    )
    bp_np = np.ascontiguousarray(
        bm.reshape(NG, N_HB, HB).transpose(1, 0, 2).reshape(N_HB, GW)
        .astype(BF16NP)
    )
    hxf = np.asarray(hx, dtype=np.float32)
    in_maps = []
    for c in range(N_CORES):
        rs = slice(c * BL, (c + 1) * BL)
        hxTc = hxf[rs].T
        in_maps.append(
            {
                "hxT": np.ascontiguousarray(hxTc.astype(BF16NP)),
                "hx8T": np.ascontiguousarray(q8(hxTc)),
                "cx1": np.ascontiguousarray(cx1[rs], dtype=np.float32),
                "cx2": np.ascontiguousarray(cx2[rs], dtype=np.float32),
                "dt": np.ascontiguousarray(dt[rs], dtype=np.float32),
                "Wa": Wa_np,
                "Wb": Wb_np,
                "W8a": W8a_np,
                "W8b": W8b_np,
                "W8d": W8d_np,
                "bp": bp_np,
            }
        )
    res = bass_utils.run_bass_kernel_spmd(
        nc, in_maps, core_ids=list(range(N_CORES)), trace=trace
    )
    out = np.concatenate([r["out"] for r in res.results], axis=1)
    if trace:
        kernel.last_exec_time_ns = res.exec_time_ns
        kernel.last_results = res
    return out


# revision 26
# speedup vs baseline: 1.0302x; 1.0302x over previous
"""Fused 7-gate continuous-time LSTM cell on 8 Trainium2 NeuronCores.

Data-parallel over the batch dim: each core gets B/8 = 1024 rows, the fused
gate weight W [2048, 7*2048] is replicated. g = hx @ W + b runs in mixed
precision, chosen so the worst-case output error stays ~27% under the 2e-2
gate (validated against the fp32 reference in numpy):
  - sigmoid gates (i1,i2,f1,f2,o): K rows 0..1535 in bf16, rows 1536..2047
    in fp8e4m3 with DoubleRow (2 MACs/cell/cycle)
  - z (tanh) gate: all bf16 (most error-sensitive)
  - d (decay) gate: all fp8 DoubleRow (its quantization error is invisible
    in the output: softplus->exp(-u*decay) damps it)
All weights are host-scaled x64 so fp8 W values sit in e4m3's normal range;
the /64 is folded into the ACT `scale` operand of the epilogue (free).
The z gate is additionally pre-scaled x2 so tanh(x) = 2*sigmoid(2x)-1 rides
the batched sigmoid chain.

Epilogue uses only Exp/Ln (single ACT table set, no ~2.7us table thrash):
  sigmoid(x) = exp(-softplus(-x)) = Exp(-Ln(1+Exp(-x)))
  E          = exp(-u*softplus(d)) via Exp(scale=-u[partition]) on Ln(1+Exp(d))

Per-block work is software-pipelined in two phases so the in-order DVE queue
never holds block i's gate arithmetic (which waits on ACT) ahead of block
i+1's PSUM-draining bias-add (which ACT i+1 needs):
  phase1(i) = matmuls, bias-add, batched sigmoid chain, E chain
  phase2(i) = gate arithmetic, tanh(c_t) chain, stores  (after phase1(i+1))
"""

import sys

sys.path.insert(0, "/opt/trn_rl_repo")

import numpy as np
import ml_dtypes

import concourse.bass as bass
import concourse.mybir as mybir
import concourse.tile as tile
from concourse import bacc, bass_utils

B, D, H, NG = 8192, 2048, 2048, 7
N_CORES = 8
BL = B // N_CORES  # 1024 rows per core
P = 128
HB = 256  # h-block per epilogue step
N_HB = H // HB  # 8
GW = NG * HB  # 1792 gate-block width per hb
KT = D // P  # 16 contraction subtiles
MT = BL // P  # 8 m-chunks per core
KBF = 12  # bf16 k-subtiles for the sigmoid gates (k < 1536)
SW = 6 * HB  # sigmoid+z block width in gsb (1536)

F32 = mybir.dt.float32
BF16 = mybir.dt.bfloat16
FP8 = mybir.dt.float8e4
AF = mybir.ActivationFunctionType
DR = mybir.MatmulPerfMode.DoubleRow
BF16NP = ml_dtypes.bfloat16
FP8NP = ml_dtypes.float8_e4m3

_cached_nc = None

# Make Exp/Ln resolvable only from the one ACT table set that holds both, so
# the table-load inserter hoists a single ACT_TABLE_LOAD instead of thrashing
# ~3 loads (~1.3us each) per block between an exp-set and an ln-set. Set order
# and count are preserved so act_func_set_id indices stay valid.
_ACT_SET = "natural_log_exp_and_others"
_tables_patched = False


def _patch_act_tables():
    global _tables_patched
    if _tables_patched:
        return
    orig = bacc.get_activation_tables

    def patched(arch):
        tabs = {k: set(v) for k, v in orig(arch).items()}
        assert _ACT_SET in tabs and {AF.Exp, AF.Ln} <= tabs[_ACT_SET], tabs.keys()
        for k in tabs:
            if k != _ACT_SET:
                tabs[k] -= {AF.Exp, AF.Ln}
        return tabs

    bacc.get_activation_tables = patched
    _tables_patched = True


def _build():
    _patch_act_tables()
    nc = bacc.Bacc("TRN2", target_bir_lowering=False, debug=False,
                   num_devices=N_CORES)
    hxT = nc.dram_tensor("hxT", [D, BL], BF16, kind="ExternalInput").ap()
    hx8T = nc.dram_tensor("hx8T", [D, BL], FP8, kind="ExternalInput").ap()
    cx1 = nc.dram_tensor("cx1", [BL, H], F32, kind="ExternalInput").ap()
    cx2 = nc.dram_tensor("cx2", [BL, H], F32, kind="ExternalInput").ap()
    dt_in = nc.dram_tensor("dt", [BL, 1], F32, kind="ExternalInput").ap()
    # bf16 weights: k<1536 x [hb, 6 gates(i1,i2,f1,f2,o,z), 256]
    Wa = nc.dram_tensor("Wa", [KBF * P, N_HB * SW], BF16,
                        kind="ExternalInput").ap()
    # bf16 o+z gate tail: k>=1536 x [hb, 512]
    Wb = nc.dram_tensor("Wb", [(KT - KBF) * P, N_HB * 512], BF16,
                        kind="ExternalInput").ap()
    # fp8 sigmoid-gate tail, one contiguous block per DR matmul slice:
    # [t'(2), hb, p, j, w] with k = 1536 + t'*256 + j*128 + p
    W8a = nc.dram_tensor("W8a", [2, N_HB, P, 2, 512], FP8,
                         kind="ExternalInput").ap()
    W8b = nc.dram_tensor("W8b", [2, N_HB, P, 2, 512], FP8,
                         kind="ExternalInput").ap()
    # fp8 d gate, all k: [t(8), hb, p, j, 256]
    W8d = nc.dram_tensor("W8d", [8, N_HB, P, 2, HB], FP8,
                         kind="ExternalInput").ap()
    bp = nc.dram_tensor("bp", [N_HB, GW], BF16, kind="ExternalInput").ap()
    out = nc.dram_tensor("out", [3, BL, H], F32, kind="ExternalOutput").ap()

    from contextlib import ExitStack

    with tile.TileContext(nc) as tc, ExitStack() as ctx:
        const_pool = ctx.enter_context(tc.tile_pool(name="const", bufs=1))
        hx_pool = ctx.enter_context(tc.tile_pool(name="hx", bufs=1))
        small_pool = ctx.enter_context(tc.tile_pool(name="small", bufs=4))
        wa_pool = ctx.enter_context(tc.tile_pool(name="wa", bufs=17))
        wb_pool = ctx.enter_context(tc.tile_pool(name="wb", bufs=6))
        w8s_pool = ctx.enter_context(tc.tile_pool(name="w8s", bufs=3))
        w8d_pool = ctx.enter_context(tc.tile_pool(name="w8d", bufs=10))
        bias_pool = ctx.enter_context(tc.tile_pool(name="bias", bufs=2))
        psum_pool = ctx.enter_context(tc.tile_pool(name="ps", bufs=4, space="PSUM"))
        gsb_pool = ctx.enter_context(tc.tile_pool(name="gsb", bufs=2))
        sgt_pool = ctx.enter_context(tc.tile_pool(name="sgt", bufs=2))
        sig_pool = ctx.enter_context(tc.tile_pool(name="sig", bufs=2))
        epi_pool = ctx.enter_context(tc.tile_pool(name="epi", bufs=2))
        cx_pool = ctx.enter_context(tc.tile_pool(name="cx", bufs=4))
        out_pool = ctx.enter_context(tc.tile_pool(name="outp", bufs=2))

        # -u per batch row (u = dt), laid out [128, m-chunk]
        negu = const_pool.tile([P, MT], F32)
        for m in range(MT):
            dtt = small_pool.tile([P, 1], F32, tag="dt")
            nc.scalar.dma_start(dtt, dt_in[m * P : (m + 1) * P, :])
            nc.vector.tensor_scalar_mul(negu[:, m : m + 1], dtt, -1.0)

        # resident hx^T in k-chunked tiles, issued in first-use order so the
        # first matmuls wait on ~1MB, not the whole 6MB
        def _hx_load(dram, dtype, base, n, tag):
            t = hx_pool.tile([P, n, BL], dtype, tag=tag, name=f"hx_{tag}")
            nc.gpsimd.dma_start(
                t,
                bass.AP(tensor=dram.tensor, offset=dram.offset + base * P * BL,
                        ap=[[BL, P], [P * BL, n], [1, BL]]),
            )
            return (base, n, t)
        hxT_tiles = [_hx_load(hxT, BF16, 0, 4, "t0")]
        hxT_tiles.append(_hx_load(hxT, BF16, 4, 8, "t4"))
        hx8_tiles = [_hx_load(hx8T, FP8, 0, 12, "e0")]
        hxT_tiles.append(_hx_load(hxT, BF16, 12, 4, "t12"))
        hx8_tiles.append(_hx_load(hx8T, FP8, 12, 4, "e12"))

        def hxT_lhs(j, ms):
            for base, n, t in hxT_tiles:
                if base <= j < base + n:
                    return t[:, j - base, ms]

        def hx8_lhs(ksub, ms):
            for base, n, t in hx8_tiles:
                if base <= ksub < base + n - 1:
                    return t[:, ksub - base : ksub - base + 2, ms]

        def phase1(hb, m, bt, wa, wb, w8s, w8d):
            ms = slice(m * P, (m + 1) * P)
            cs = slice(hb * HB, (hb + 1) * HB)
            cx1t = cx_pool.tile([P, HB], F32, tag="cx1")
            nc.gpsimd.dma_start(cx1t, cx1[ms, cs])
            cx2t = cx_pool.tile([P, HB], F32, tag="cx2")
            nc.gpsimd.dma_start(cx2t, cx2[ms, cs])

            ps0 = psum_pool.tile([P, 1024], F32, tag="ps", name=f"ps0_{hb}_{m}")
            ps1 = psum_pool.tile([P, 1024], F32, tag="ps", name=f"ps1_{hb}_{m}")
            # gsb col ranges: ps0 -> [i1,i2,f1,f2], ps1[0:256] -> o,
            # ps1[256:512] -> z, ps1[512:768] -> d
            for j in range(KBF):
                lhs = hxT_lhs(j, ms)
                st = j == 0
                nc.tensor.matmul(ps0[:, 0:512], lhs, wa[j][:, 0:512],
                                 start=st, stop=False)
                nc.tensor.matmul(ps0[:, 512:1024], lhs, wa[j][:, 512:1024],
                                 start=st, stop=False)
                nc.tensor.matmul(ps1[:, 0:512], lhs, wa[j][:, 1024:1536],
                                 start=st, stop=False)
            for j in range(KBF, KT):  # o+z gate bf16 tail
                nc.tensor.matmul(ps1[:, 0:512], hxT_lhs(j, ms),
                                 wb[j - KBF][:], start=False, stop=(j == KT - 1))
            for t in range(6):  # d-gate fp8 DoubleRow (grouped, not interleaved)
                l8 = hx8_lhs(2 * t, ms)
                nc.tensor.matmul(ps1[:, 512:768], l8, w8d[t][:],
                                 start=(t == 0), stop=False, perf_mode=DR)
            for t in (6, 7):  # i1,i2,f1,f2 + d-gate fp8 DoubleRow tail
                l8 = hx8_lhs(2 * t, ms)
                last = t == 7
                w8at, w8bt = w8s[t - 6]
                nc.tensor.matmul(ps0[:, 0:512], l8, w8at[:],
                                 start=False, stop=last, perf_mode=DR)
                nc.tensor.matmul(ps0[:, 512:1024], l8, w8bt[:],
                                 start=False, stop=last, perf_mode=DR)
                nc.tensor.matmul(ps1[:, 512:768], l8, w8d[t][:],
                                 start=False, stop=last, perf_mode=DR)

            # bias add drains PSUM; gsb = 64*(g+b) (weights/bias host-scaled)
            gsb = gsb_pool.tile([P, GW], F32, tag="gsb")
            nc.vector.tensor_add(gsb[:, 0:1024], ps0[:], bt[:, 0:1024])
            nc.vector.tensor_add(gsb[:, 1024:GW], ps1[:, 0:768], bt[:, 1024:GW])

            # 5 sigmoid gates + pre-scaled z gate batched 1536 wide;
            # the 1/64 unscale rides the first Exp: sig = Exp(-Ln(1+Exp(-x/64)))
            e1 = sgt_pool.tile([P, SW], F32, tag="sgt")
            nc.scalar.activation(e1, gsb[:, 0:SW], AF.Exp, scale=-1.0 / 64)
            sp = sgt_pool.tile([P, SW], F32, tag="sgt")
            nc.scalar.activation(sp, e1, AF.Ln, bias=1.0)
            sig = sig_pool.tile([P, SW], F32, tag="sig")
            nc.scalar.activation(sig, sp, AF.Exp, scale=-1.0)

            # E = exp(-u * softplus(d))
            ed = epi_pool.tile([P, HB], F32, tag="ed")
            nc.scalar.activation(ed, gsb[:, 6 * HB : 7 * HB], AF.Exp,
                                 scale=1.0 / 64)
            spd = epi_pool.tile([P, HB], F32, tag="spd")
            nc.scalar.activation(spd, ed, AF.Ln, bias=1.0)
            E = epi_pool.tile([P, HB], F32, tag="E")
            nc.scalar.activation(E, spd, AF.Exp, scale=negu[:, m : m + 1])

            return dict(ms=ms, cs=cs, cx1t=cx1t, cx2t=cx2t, sig=sig, E=E)

        def phase2(st):
            sig, E = st["sig"], st["E"]
            i1 = sig[:, 0:HB]
            i2 = sig[:, HB : 2 * HB]
            f1 = sig[:, 2 * HB : 3 * HB]
            f2 = sig[:, 3 * HB : 4 * HB]
            o = sig[:, 4 * HB : 5 * HB]
            # z = tanh = 2*sigmoid(2x)-1 (the x2 was folded into W/b on host)
            z = epi_pool.tile([P, HB], F32, tag="z")
            nc.vector.tensor_scalar(
                out=z, in0=sig[:, 5 * HB : 6 * HB], scalar1=2.0, scalar2=-1.0,
                op0=mybir.AluOpType.mult, op1=mybir.AluOpType.add,
            )

            t1 = epi_pool.tile([P, HB], F32, tag="t1")
            nc.vector.tensor_mul(t1, f1, st["cx1t"])
            t2 = epi_pool.tile([P, HB], F32, tag="t2")
            nc.vector.tensor_mul(t2, i1, z)
            cy1 = out_pool.tile([P, HB], F32, tag="cy1")
            nc.vector.tensor_add(cy1, t1, t2)

            t3 = epi_pool.tile([P, HB], F32, tag="t3")
            nc.vector.tensor_mul(t3, f2, st["cx2t"])
            t4 = epi_pool.tile([P, HB], F32, tag="t4")
            nc.vector.tensor_mul(t4, i2, z)
            cy2 = out_pool.tile([P, HB], F32, tag="cy2")
            nc.vector.tensor_add(cy2, t3, t4)

            dif = epi_pool.tile([P, HB], F32, tag="dif")
            nc.vector.tensor_sub(dif, cy1, cy2)
            t5 = epi_pool.tile([P, HB], F32, tag="t5")
            nc.vector.tensor_mul(t5, dif, E)
            ct = epi_pool.tile([P, HB], F32, tag="ct")
            nc.vector.tensor_add(ct, cy2, t5)

            # tanh(ct) = 2*sigmoid(2*ct) - 1
            ec = epi_pool.tile([P, HB], F32, tag="ec")
            nc.scalar.activation(ec, ct, AF.Exp, scale=-2.0)
            spc = epi_pool.tile([P, HB], F32, tag="spc")
            nc.scalar.activation(spc, ec, AF.Ln, bias=1.0)
            s2c = epi_pool.tile([P, HB], F32, tag="s2c")
            nc.scalar.activation(s2c, spc, AF.Exp, scale=-1.0)
            tct = epi_pool.tile([P, HB], F32, tag="tct")
            nc.vector.tensor_scalar(
                out=tct, in0=s2c, scalar1=2.0, scalar2=-1.0,
                op0=mybir.AluOpType.mult, op1=mybir.AluOpType.add,
            )
            ht = out_pool.tile([P, HB], F32, tag="ht")
            nc.vector.tensor_mul(ht, o, tct)

            nc.gpsimd.dma_start(out[0, st["ms"], st["cs"]], cy1)
            nc.gpsimd.dma_start(out[1, st["ms"], st["cs"]], cy2)
            nc.gpsimd.dma_start(out[2, st["ms"], st["cs"]], ht)

        def issue_hb_loads(hb):
            bsl = bp[hb, :]
            b_bcast = bass.AP(
                tensor=bsl.tensor, offset=bsl.offset, ap=[[0, P], *bsl.ap]
            )
            bt = bias_pool.tile([P, GW], BF16, tag="bt", name=f"bt_{hb}")
            nc.gpsimd.dma_start(bt, b_bcast)
            wa, wb, w8s, w8d = [], [], [], []
            for j in range(KT - KBF):
                wt = wb_pool.tile([P, 512], BF16, tag="wb", name=f"wb_{hb}_{j}")
                nc.sync.dma_start(
                    wt, Wb[j * P : (j + 1) * P, hb * 512 : (hb + 1) * 512]
                )
                wb.append(wt)
            for t in range(2):
                w8at = w8s_pool.tile([P, 2, 512], FP8, tag="w8a",
                                     name=f"w8a_{hb}_{t}")
                nc.sync.dma_start(w8at, W8a[t, hb])
                w8bt = w8s_pool.tile([P, 2, 512], FP8, tag="w8b",
                                     name=f"w8b_{hb}_{t}")
                nc.sync.dma_start(w8bt, W8b[t, hb])
                w8s.append((w8at, w8bt))
            for t in range(8):
                w8dt = w8d_pool.tile([P, 2, HB], FP8, tag="w8d",
                                     name=f"w8d_{hb}_{t}")
                nc.sync.dma_start(w8dt, W8d[t, hb])
                w8d.append(w8dt)
            return bt, wb, w8s, w8d

        def issue_wa_tiles(hb, js):
            res = []
            for j in js:
                wt = wa_pool.tile([P, SW], BF16, tag="wa", name=f"wa_{hb}_{j}")
                nc.sync.dma_start(
                    wt, Wa[j * P : (j + 1) * P, hb * SW : (hb + 1) * SW]
                )
                res.append(wt)
            return res

        pending = None
        wa_head = issue_wa_tiles(0, range(4))
        for hb in range(N_HB):
            wa = wa_head + issue_wa_tiles(hb, range(4, KBF))
            bt, wb, w8s, w8d = issue_hb_loads(hb)
            for m in range(MT):
                st = phase1(hb, m, bt, wa, wb, w8s, w8d)
                if m == 6 and hb + 1 < N_HB:
                    wa_head = issue_wa_tiles(hb + 1, range(4))
                if pending is not None:
                    phase2(pending)
                pending = st
        phase2(pending)

    nc.compile()
    return nc


def _get_nc():
    global _cached_nc
    if _cached_nc is None:
        _cached_nc = _build()
    return _cached_nc


def kernel(hx, cx1, cx2, tj, dt, W, b, trace=False):
    nc = _get_nc()
    Wm = np.asarray(W, dtype=np.float32).copy()
    bm = np.asarray(b, dtype=np.float32).reshape(NG * H).copy()
    # fold tanh(x) = 2*sigmoid(2x)-1: pre-scale z-gate columns by 2
    Wm[:, 5 * H : 6 * H] *= 2.0
    bm[5 * H : 6 * H] *= 2.0
    # global x64 so fp8 weights sit in e4m3's normal range; /64 folded into
    # the epilogue ACT scale
    Wm *= 64.0
    bm *= 64.0
    W4 = Wm.reshape(D, NG, N_HB, HB)  # [k, gate, hb, col]
    Wa_np = np.ascontiguousarray(
        W4[: KBF * P, :6].transpose(0, 2, 1, 3).reshape(KBF * P, N_HB * SW)
        .astype(BF16NP)
    )
    Wb_np = np.ascontiguousarray(
        W4[KBF * P :, 4:6].transpose(0, 2, 1, 3)
        .reshape((KT - KBF) * P, N_HB * 512).astype(BF16NP)
    )
    q8 = lambda a: np.clip(a, -240, 240).astype(FP8NP)
    W8s_full = q8(W4[KBF * P :, :4]
                  .transpose(0, 2, 1, 3)           # [k', hb, gate, col]
                  .reshape(2, 2, P, N_HB, 4 * HB)  # [t', j, p, hb, c]
                  .transpose(0, 3, 2, 1, 4))       # [t', hb, p, j, c]
    W8a_np = np.ascontiguousarray(W8s_full[..., 0:512])
    W8b_np = np.ascontiguousarray(W8s_full[..., 512:1024])
    W8d_np = np.ascontiguousarray(
        q8(W4[:, 6]
           .reshape(8, 2, P, N_HB, HB)        # [t, j, p, hb, c]
           .transpose(0, 3, 2, 1, 4))         # [t, hb, p, j, c]
    )
    bp_np = np.ascontiguousarray(
        bm.reshape(NG, N_HB, HB).transpose(1, 0, 2).reshape(N_HB, GW)
        .astype(BF16NP)
    )
    hxf = np.asarray(hx, dtype=np.float32)
    in_maps = []
    for c in range(N_CORES):
        rs = slice(c * BL, (c + 1) * BL)
        hxTc = hxf[rs].T
        in_maps.append(
            {
                "hxT": np.ascontiguousarray(hxTc.astype(BF16NP)),
                "hx8T": np.ascontiguousarray(q8(hxTc)),
                "cx1": np.ascontiguousarray(cx1[rs], dtype=np.float32),
                "cx2": np.ascontiguousarray(cx2[rs], dtype=np.float32),
                "dt": np.ascontiguousarray(dt[rs], dtype=np.float32),
                "Wa": Wa_np,
                "Wb": Wb_np,
                "W8a": W8a_np,
                "W8b": W8b_np,
                "W8d": W8d_np,
                "bp": bp_np,
            }
        )
    res = bass_utils.run_bass_kernel_spmd(
        nc, in_maps, core_ids=list(range(N_CORES)), trace=trace
    )
    out = np.concatenate([r["out"] for r in res.results], axis=1)
    if trace:
        kernel.last_exec_time_ns = res.exec_time_ns
        kernel.last_results = res
    return out
